# revision 26
# baseline (speedup 1.0000x reference)
"""Trainium2 Bass kernel for nn_FCond (FiLM-conditioned MLP chain).

Reference computation (B=32, N=100000, D=3, CDIM=128):
    h = x
    for kblk in [0, 1, 2, 2, 2, 2]:
        h = tanh((h @ Wk.T + bk) * sigmoid(c @ Wsk.T + bsk) + (c @ Wbk.T + bbk))

Since the FiLM conditioning depends only on (c, weights), each (batch,
block) reduces to an affine map  h' = tanh(A_kb @ h + d_kb)  with
A_kb [3,3], d_kb [3] precomputed on the host in float64.

Device strategy (pure data parallel over 8 cores, 4 batches/core):
  - Layout: partition p = b*32 + comp*10 + g  (4 batch-bands of 32
    partitions; 3 comps x 10 point-groups per band). Row 30 of each
    band is a constant-1.0 row, row 31 zero padding.
  - Each block is ONE block-diagonal [128x128] fp16 matmul on TensorE
    (40 real points per column), PSUM f32, then ScalarE does
    tanh(psum), evacuating PSUM->SBUF as fp16.
  - The affine bias d rides inside the matmul: weight column p gets
    d[p] in the ones-row, and the ones-row regenerates itself through
    every block via W[ones,ones]=16 (tanh(16) == 1.0 in fp16). No bias
    DMA, no per-partition bias operand in the activation.
  - Hand-scheduled engine programs (no TileContext), 30 stages
    s=(kblk, chunk) over 5 UNIFORM 2000-column chunks rotating in
    groups (0,1,2)/(3,4): uniform sizes keep the PE (1 cyc/col at its
    sustained 1.2 GHz p-state) and ACT (1 elem/cyc/lane, 1.2 GHz) in
    lockstep with ~100ns/stage of PE margin, so ACT — the roofline
    engine — never stalls. PSUM ping-pongs 2x[128,2048] (4 banks
    each); a single act_sem >= s-1 wait on the PE covers both the
    input dependency (ACT stage s-3 or s-2) and the PSUM WAR (s-2).
  - DMA: per-engine DGE rings are only ~125 GB/s, so transfers spread
    across the sync, vector and gpsimd queues (x chunk 0 split in
    half across two rings to cut the pipeline-fill latency). Outputs
    stream out per chunk as soon as the last block's tanh lands; the
    final chunk is split in half across two rings to hide the tail.

Numerics: weights/bias/activations fp16 (PE @ 1 cyc/col), PSUM f32,
tanh on ACT exact. Measured end-to-end rel err vs the fp32 reference:
~4e-4.
"""
import sys
import types

import numpy as np

B, N, D, CDIM = 32, 100000, 3, 128
NCORES = 8
BPC = B // NCORES          # batches per core
G = 10                     # point-groups per (batch, comp)
L = 10000                  # points per partition stream (N / G, exact)
P = 128                    # partitions
MM_F = 512                 # matmul free chunk (1 PSUM f32 bank)

# Chunk sizes: uniform inside each rotation group, and only DOWNSIZE
# transitions between groups (PE stage s races ACT stage s-1 in
# lockstep; an upsize transition would stall ACT). The smaller last
# group also shrinks the tail's exposed output DMA bytes.
CHUNKS = (2048, 2048, 2048, 1928, 1928)   # sum == L
NCHUNK = 5
GROUPS = ((0, 1, 2), (3, 4))
WSETS = (0, 1, 2, 2, 2, 2)

PROFILE = False            # set by test harness; collects HW exec time
LAST_EXEC_NS = None

_CACHE = {}


def _install_profile_shim():
    """Register the NTFF profile hook (missing antenv.axon_hooks in this
    container) so run_bass_kernel_spmd(trace=True) can report exec time."""
    if "antenv.axon_hooks" in sys.modules:
        return
    mod = types.ModuleType("antenv.axon_hooks")
    _state = {"hook": None}
    mod.set_axon_ntff_profile_hook = lambda h: _state.__setitem__("hook", h)
    mod.get_axon_ntff_profile_hook = lambda: _state["hook"]
    sys.modules["antenv.axon_hooks"] = mod
    try:
        from trn_agent_boot.trn_boot import _ntff_profile_via_ctypes
        mod.set_axon_ntff_profile_hook(
            _ntff_profile_via_ctypes("/opt/axon/libaxon_pjrt.so"))
    except Exception:
        pass
    import concourse.bass_utils as bu
    bu.upload_artifacts = lambda tmpdir: f"local:{tmpdir}"


def _build_program():
    import concourse.bass as bass
    import concourse.bacc as bacc
    from concourse import mybir

    f32 = mybir.dt.float32
    f16 = mybir.dt.float16
    Tanh = mybir.ActivationFunctionType.Tanh

    nc = bacc.Bacc("TRN2", target_bir_lowering=False, debug=False)
    x_d = nc.declare_dram_parameter("x", [P, L], f16, isOutput=False)
    w_d = nc.declare_dram_parameter("w", [P, 3 * P], f16, isOutput=False)
    y_d = nc.declare_dram_parameter("y", [P, L], f16, isOutput=True)

    offs = [sum(CHUNKS[:i]) for i in range(NCHUNK)]

    # static SBUF tensors
    xin = [nc.alloc_sbuf_tensor(f"xin{c}", [P, CHUNKS[c]], f16).ap()
           for c in range(NCHUNK)]
    ha = [nc.alloc_sbuf_tensor(f"ha{c}", [P, CHUNKS[c]], f16).ap()
          for c in range(NCHUNK)]
    hb = [nc.alloc_sbuf_tensor(f"hb{c}", [P, CHUNKS[c]], f16).ap()
          for c in range(NCHUNK)]
    yout = [nc.alloc_sbuf_tensor(f"yout{c}", [P, CHUNKS[c]], f16).ap()
            for c in range(NCHUNK)]
    wall = nc.alloc_sbuf_tensor("wall", [P, 3 * P], f16).ap()
    ps = [nc.alloc_psum_tensor(f"ps{i}", [P, 2048], f32).ap()
          for i in range(2)]

    # stage enumeration: rotation inside each group gives every stage's
    # input (ACT stage s-3 / s-2) and PSUM WAR (ACT s-2) a single
    # act_sem >= s-1 wait.
    stages = []
    for grp in GROUPS:
        for kblk in range(6):
            for ci in grp:
                stages.append((kblk, ci))
    idx = {kc: s for s, kc in enumerate(stages)}
    NST = len(stages)
    LAST = stages[-1]
    TH = (CHUNKS[LAST[1]] // 2 + 3) & ~3   # last-stage tanh split column

    def h_in(kblk, ci):
        if kblk == 0:
            return xin[ci]
        return ha[ci] if kblk % 2 == 1 else hb[ci]

    def h_out(kblk, ci):
        if kblk == 5:
            return yout[ci]
        return ha[ci] if kblk % 2 == 0 else hb[ci]

    with (
        nc.Block(no_gpsimd_drain=True) as block,
        nc.semaphore("dma_x0") as dx0,
        nc.semaphore("dma_x1") as dx1,
        nc.semaphore("dma_x2") as dx2,
        nc.semaphore("dma_x3") as dx3,
        nc.semaphore("dma_x4") as dx4,
        nc.semaphore("dma_w") as dww,
        nc.semaphore("dma_out") as dout,
        nc.semaphore("act_sem") as act_sem,
        nc.semaphore("pe_sem") as pe_sem,
    ):
        dxs = [dx0, dx1, dx2, dx3, dx4]

        def x_dma(eng, ci, p0, p1):
            eng.dma_start(out=xin[ci][p0:p1, :],
                          in_=x_d[p0:p1, offs[ci]:offs[ci] + CHUNKS[ci]]
                          ).then_inc(dxs[ci], 16)

        def y_dma(eng, ci, req, p0=0, p1=P):
            eng.dma_start(out=y_d[p0:p1, offs[ci]:offs[ci] + CHUNKS[ci]],
                          in_=yout[ci][p0:p1, :]
                          )._wait_ge(act_sem, req).then_inc(dout, 16)

        # DMA rings are DESCRIPTOR-bound: a [128, anything] transfer costs
        # ~3.4us on the sync/scalar rings and ~8us on the gpsimd ring,
        # almost independent of column count (one descriptor per partition
        # line). So fill/tail-critical transfers (w, x0, x1, y4) are split
        # in HALF BY PARTITION across the sync+scalar rings (64
        # descriptors each ~1.7us); slack-rich bulk (x2, x4, y0-y2) rides
        # the slow gpsimd ring.
        H = P // 2

        @block.gpsimd
        def _(g: bass.BassEngine):
            # keep the slow gpsimd ring (and the shared fabric) clear of
            # the fill-critical w/x0/x1 window
            g.wait_ge(dxs[0], 32)
            x_dma(g, 2, H, P)
            g.wait_ge(dxs[1], 32)
            x_dma(g, 4, 0, P)
            y_dma(g, 0, idx[(5, 0)] + 1)
            y_dma(g, 1, idx[(5, 1)] + 1)
            y_dma(g, 3, idx[(5, 3)] + 1, H, P)
            # bottom-left quarter of the final chunk (cols 0:TH ready
            # after the first half of the split last tanh)
            g.dma_start(out=y_d[H:P, offs[4]:offs[4] + TH],
                        in_=yout[4][H:P, 0:TH]
                        )._wait_ge(act_sem, NST).then_inc(dout, 16)

        @block.sync
        def _(sync: bass.BassEngine):
            x_dma(sync, 0, 0, H)
            sync.dma_start(out=wall[0:H, :], in_=w_d[0:H, :]
                           ).then_inc(dww, 16)
            x_dma(sync, 1, 0, H)
            x_dma(sync, 2, 0, H)
            x_dma(sync, 3, 0, P)
            y_dma(sync, 2, idx[(5, 2)] + 1)
            y_dma(sync, 3, idx[(5, 3)] + 1, 0, H)
            # final chunk, top partitions: first-half columns as soon as
            # the split tanh's first half lands, then the second half
            sync.dma_start(out=y_d[0:H, offs[4]:offs[4] + TH],
                           in_=yout[4][0:H, 0:TH]
                           )._wait_ge(act_sem, NST).then_inc(dout, 16)
            sync.dma_start(out=y_d[0:H, offs[4] + TH:offs[4] + CHUNKS[4]],
                           in_=yout[4][0:H, TH:CHUNKS[4]]
                           )._wait_ge(act_sem, NST + 1).then_inc(dout, 16)
            sync.wait_ge(dout, 16 * 9)

        @block.scalar
        def _(scalar: bass.BassEngine):
            # bottom halves of x0/w/x1 ride the otherwise-idle ACT queue;
            # their issue cost lands during pipeline fill
            x_dma(scalar, 0, H, P)
            scalar.dma_start(out=wall[H:P, :], in_=w_d[H:P, :]
                             ).then_inc(dww, 16)
            x_dma(scalar, 1, H, P)
            for s, (kblk, ci) in enumerate(stages):
                if (kblk, ci) == LAST:
                    # split the final tanh by column so its output DMAs
                    # start ~1us earlier (they are partition-split
                    # quarters on the sync/gpsimd/scalar rings)
                    for a, b_ in ((0, TH), (TH, CHUNKS[ci])):
                        act = scalar.activation(
                            h_out(kblk, ci)[:, a:b_], ps[s % 2][:, a:b_],
                            Tanh, bias=0.0, scale=1.0)
                        if a == 0:
                            act._wait_ge(pe_sem, s + 1)
                        act.then_inc(act_sem, 1)
                else:
                    scalar.activation(
                        h_out(kblk, ci), ps[s % 2][:, 0:CHUNKS[ci]], Tanh,
                        bias=0.0, scale=1.0,
                    )._wait_ge(pe_sem, s + 1).then_inc(act_sem, 1)
            # bottom-right quarter of the final output on the now-idle
            # ACT queue (the act_sem wait makes the DMA observe the
            # tanh's posted SBUF writes - program order alone does not
            # order posted writes against another engine's reads)
            scalar.dma_start(
                out=y_d[H:P, offs[4] + TH:offs[4] + CHUNKS[4]],
                in_=yout[4][H:P, TH:CHUNKS[4]]
                )._wait_ge(act_sem, NST + 1).then_inc(dout, 16)

        @block.tensor
        def _(tensor: bass.BassEngine):
            tensor.wait_ge(dww, 32)
            for s, (kblk, ci) in enumerate(stages):
                ks = WSETS[kblk]
                if kblk == 0:
                    tensor.wait_ge(dxs[ci], 32 if ci <= 2 else 16)
                rhs = h_in(kblk, ci)
                for j in range(0, CHUNKS[ci], MM_F):
                    je = min(j + MM_F, CHUNKS[ci])
                    mm = tensor.matmul(ps[s % 2][:, j:je],
                                       wall[:, ks * P:(ks + 1) * P],
                                       rhs[:, j:je],
                                       start=True, stop=True)
                    if j == 0 and s >= 2:
                        # input-ready + psum WAR, folded into one wait
                        mm._wait_ge(act_sem, s - 1)
                mm.then_inc(pe_sem, 1)

    nc.compile()
    return nc


def _film_params(c, Wk, bk, Wsk, bsk, Wbk, bbk):
    """A[b] = diag(scale[b]) @ Wk ; d[b] = scale[b]*bk + shift[b], float64."""
    c = c.astype(np.float64)
    scale = 1.0 / (1.0 + np.exp(-(c @ Wsk.astype(np.float64).T
                                  + bsk.astype(np.float64))))     # [B,3]
    shift = c @ Wbk.astype(np.float64).T + bbk.astype(np.float64)  # [B,3]
    A = scale[:, :, None] * Wk.astype(np.float64)[None]            # [B,3,3]
    d = scale * bk.astype(np.float64) + shift                      # [B,3]
    return A, d


def kernel(t, x, c,
           W0, b0, Ws0, bs0, Wb0, bb0,
           W1, b1, Ws1, bs1, Wb1, bb1,
           W2, b2, Ws2, bs2, Wb2, bb2):
    global LAST_EXEC_NS
    if PROFILE:
        _install_profile_shim()
    from concourse.bass_utils import run_bass_kernel_spmd

    x = np.asarray(x)
    c = np.asarray(c)
    (W0, b0, Ws0, bs0, Wb0, bb0, W1, b1, Ws1, bs1, Wb1, bb1,
     W2, b2, Ws2, bs2, Wb2, bb2) = (
        np.asarray(a) for a in (W0, b0, Ws0, bs0, Wb0, bb0,
                                W1, b1, Ws1, bs1, Wb1, bb1,
                                W2, b2, Ws2, bs2, Wb2, bb2))
    out_dtype = x.dtype

    if "prog" not in _CACHE:
        _CACHE["prog"] = _build_program()
    nc = _CACHE["prog"]

    # ---- host: FiLM affine params per (weight-set, batch), float64 ----
    sets = [
        _film_params(c, W0, b0, Ws0, bs0, Wb0, bb0),
        _film_params(c, W1, b1, Ws1, bs1, Wb1, bb1),
        _film_params(c, W2, b2, Ws2, bs2, Wb2, bb2),
    ]

    # ---- host: shard + relayout x ----
    # [B, N, 3] -> per core [128, L] fp16: p = b*32 + comp*10 + g
    xp = np.ascontiguousarray(x, dtype=np.float32)
    # [B, 3, G, L]
    xt = np.ascontiguousarray(xp.transpose(0, 2, 1)).reshape(B, D, G, L)

    in_maps = []
    for cc in range(NCORES):
        bs = range(cc * BPC, (cc + 1) * BPC)
        X = np.zeros((BPC, 32, L), np.float16)
        for i, b in enumerate(bs):
            X[i, :30] = xt[b].reshape(30, L)
            X[i, 30] = 1.0          # ones-row: carries the bias via matmul
        W6 = np.zeros((P, 3 * P), np.float16)
        for k in range(3):
            A, dv = sets[k]
            for i, b in enumerate(bs):
                ones_r = i * 32 + 30
                for ci_ in range(3):
                    for cj in range(3):
                        a = np.float16(A[b, ci_, cj])
                        for g in range(G):
                            W6[i * 32 + cj * G + g,
                               k * P + i * 32 + ci_ * G + g] = a
                    # bias d rides the ones-row
                    W6[ones_r, k * P + i * 32 + ci_ * G:
                       k * P + i * 32 + ci_ * G + G] = np.float16(dv[b, ci_])
                # ones-row regenerates itself: tanh(16.0) == 1.0 in fp16
                W6[ones_r, k * P + ones_r] = np.float16(16.0)
        in_maps.append({"x": X.reshape(P, L), "w": W6})

    res = run_bass_kernel_spmd(nc, in_maps, list(range(NCORES)),
                               trace=bool(PROFILE))
    if PROFILE:
        LAST_EXEC_NS = res.exec_time_ns

    # ---- host: gather + inverse layout ----
    out = np.empty((B, N, D), out_dtype)
    for cc in range(NCORES):
        Y = res.results[cc]["y"].reshape(BPC, 32, L)
        for i in range(BPC):
            b = cc * BPC + i
            # [30, L] -> [3, N] -> [N, 3]
            yb = Y[i, :30].reshape(D, N)
            out[b] = yb.T.astype(out_dtype, copy=False)
    return out


# revision 29
# speedup vs baseline: 1.0296x; 1.0296x over previous
"""Trainium2 Bass kernel for nn_FCond (FiLM-conditioned MLP chain).

Reference computation (B=32, N=100000, D=3, CDIM=128):
    h = x
    for kblk in [0, 1, 2, 2, 2, 2]:
        h = tanh((h @ Wk.T + bk) * sigmoid(c @ Wsk.T + bsk) + (c @ Wbk.T + bbk))

Since the FiLM conditioning depends only on (c, weights), each (batch,
block) reduces to an affine map  h' = tanh(A_kb @ h + d_kb)  with
A_kb [3,3], d_kb [3] precomputed on the host in float64.

Device strategy (pure data parallel over 8 cores, 4 batches/core):
  - Layout: partition p = b*32 + comp*10 + g  (4 batch-bands of 32
    partitions; 3 comps x 10 point-groups per band). Row 30 of each
    band is a constant-1.0 row, row 31 zero padding.
  - Each block is ONE block-diagonal [128x128] fp16 matmul on TensorE
    (40 real points per column), PSUM f32, then ScalarE does
    tanh(psum), evacuating PSUM->SBUF as fp16.
  - The affine bias d rides inside the matmul: weight column p gets
    d[p] in the ones-row, and the ones-row regenerates itself through
    every block via W[ones,ones]=16 (tanh(16) == 1.0 in fp16). No bias
    DMA, no per-partition bias operand in the activation.
  - Hand-scheduled engine programs (no TileContext), 30 stages
    s=(kblk, chunk) over 5 UNIFORM 2000-column chunks rotating in
    groups (0,1,2)/(3,4): uniform sizes keep the PE (1 cyc/col at its
    sustained 1.2 GHz p-state) and ACT (1 elem/cyc/lane, 1.2 GHz) in
    lockstep with ~100ns/stage of PE margin, so ACT — the roofline
    engine — never stalls. PSUM ping-pongs 2x[128,2048] (4 banks
    each); a single act_sem >= s-1 wait on the PE covers both the
    input dependency (ACT stage s-3 or s-2) and the PSUM WAR (s-2).
  - DMA: per-engine DGE rings are only ~125 GB/s, so transfers spread
    across the sync, vector and gpsimd queues (x chunk 0 split in
    half across two rings to cut the pipeline-fill latency). Outputs
    stream out per chunk as soon as the last block's tanh lands; the
    final chunk is split in half across two rings to hide the tail.

Numerics: weights/bias/activations fp16 (PE @ 1 cyc/col), PSUM f32,
tanh on ACT exact. Measured end-to-end rel err vs the fp32 reference:
~4e-4.
"""
import sys
import types

import numpy as np

B, N, D, CDIM = 32, 100000, 3, 128
NCORES = 8
BPC = B // NCORES          # batches per core
P = 128                    # partitions
MM_F = 512                 # matmul free chunk (1 PSUM f32 bank)

# 42 streams of 3 comps = 126 partitions (+ ones-row 126, zero-row
# 127). Stream length LC = ceil(4*100000/42): each batch owns 10 full
# streams + half of a shared boundary stream, which switches batch (and
# therefore weight block) at column BSPLIT - handled by a second weight
# "zone" and a matmul split at that column.
NST_R = 42                 # streams per core
LC = 9524                  # columns (points per stream)
BSPLIT = 4760              # boundary-stream batch-switch column
NFULL = 95240              # points covered by the 10 full streams

# Chunk sizes: uniform inside each rotation group, and only DOWNSIZE
# transitions between groups (PE stage s races ACT stage s-1 in
# lockstep; an upsize transition would stall ACT). The smaller last
# group also shrinks the tail's exposed output DMA bytes.
CHUNKS = (2048, 2048, 2048, 1690, 1690)   # sum == LC
NCHUNK = 5
GROUPS = ((0, 1, 2), (3, 4))
WSETS = (0, 1, 2, 2, 2, 2)

# Per-chunk matmul plan: (j0, j1, zone). Zone 0 weights apply to columns
# < BSPLIT (global), zone 1 after; only the two boundary streams differ
# between zones. The boundary falls inside chunk 2 at local column 664.
_Z0 = tuple((j, j + 512, 0) for j in range(0, 2048, 512))
# every matmul's output must stay inside one 512-col PSUM bank
_Z2 = ((0, 512, 0), (512, 664, 0), (664, 1024, 1), (1024, 1536, 1),
       (1536, 2048, 1))
_Z34 = ((0, 512, 1), (512, 1024, 1), (1024, 1536, 1), (1536, 1690, 1))
MMPLAN = (_Z0, _Z0, _Z2, _Z34, _Z34)

PROFILE = False            # set by test harness; collects HW exec time
LAST_EXEC_NS = None

_CACHE = {}


def _install_profile_shim():
    """Register the NTFF profile hook (missing antenv.axon_hooks in this
    container) so run_bass_kernel_spmd(trace=True) can report exec time."""
    if "antenv.axon_hooks" in sys.modules:
        return
    mod = types.ModuleType("antenv.axon_hooks")
    _state = {"hook": None}
    mod.set_axon_ntff_profile_hook = lambda h: _state.__setitem__("hook", h)
    mod.get_axon_ntff_profile_hook = lambda: _state["hook"]
    sys.modules["antenv.axon_hooks"] = mod
    try:
        from trn_agent_boot.trn_boot import _ntff_profile_via_ctypes
        mod.set_axon_ntff_profile_hook(
            _ntff_profile_via_ctypes("/opt/axon/libaxon_pjrt.so"))
    except Exception:
        pass
    import concourse.bass_utils as bu
    bu.upload_artifacts = lambda tmpdir: f"local:{tmpdir}"


def _build_program():
    import concourse.bass as bass
    import concourse.bacc as bacc
    from concourse import mybir

    f32 = mybir.dt.float32
    f16 = mybir.dt.float16
    Tanh = mybir.ActivationFunctionType.Tanh

    nc = bacc.Bacc("TRN2", target_bir_lowering=False, debug=False)
    x_d = nc.declare_dram_parameter("x", [P, LC], f16, isOutput=False)
    w_d = nc.declare_dram_parameter("w", [P, 6 * P], f16, isOutput=False)
    y_d = nc.declare_dram_parameter("y", [P, LC], f16, isOutput=True)

    offs = [sum(CHUNKS[:i]) for i in range(NCHUNK)]

    # static SBUF tensors
    xin = [nc.alloc_sbuf_tensor(f"xin{c}", [P, CHUNKS[c]], f16).ap()
           for c in range(NCHUNK)]
    ha = [nc.alloc_sbuf_tensor(f"ha{c}", [P, CHUNKS[c]], f16).ap()
          for c in range(NCHUNK)]
    hb = [nc.alloc_sbuf_tensor(f"hb{c}", [P, CHUNKS[c]], f16).ap()
          for c in range(NCHUNK)]
    yout = [nc.alloc_sbuf_tensor(f"yout{c}", [P, CHUNKS[c]], f16).ap()
            for c in range(NCHUNK)]
    wall = nc.alloc_sbuf_tensor("wall", [P, 6 * P], f16).ap()
    ps = [nc.alloc_psum_tensor(f"ps{i}", [P, 2048], f32).ap()
          for i in range(2)]

    # stage enumeration: rotation inside each group gives every stage's
    # input (ACT stage s-3 / s-2) and PSUM WAR (ACT s-2) a single
    # act_sem >= s-1 wait.
    stages = []
    for grp in GROUPS:
        for kblk in range(6):
            for ci in grp:
                stages.append((kblk, ci))
    idx = {kc: s for s, kc in enumerate(stages)}
    NST = len(stages)
    LAST = stages[-1]
    TH = (CHUNKS[LAST[1]] // 2 + 3) & ~3   # last-stage tanh split column

    def h_in(kblk, ci):
        if kblk == 0:
            return xin[ci]
        return ha[ci] if kblk % 2 == 1 else hb[ci]

    def h_out(kblk, ci):
        if kblk == 5:
            return yout[ci]
        return ha[ci] if kblk % 2 == 0 else hb[ci]

    with (
        nc.Block(no_gpsimd_drain=True) as block,
        nc.semaphore("dma_x0") as dx0,
        nc.semaphore("dma_x1") as dx1,
        nc.semaphore("dma_x2") as dx2,
        nc.semaphore("dma_x3") as dx3,
        nc.semaphore("dma_x4") as dx4,
        nc.semaphore("dma_w") as dww,
        nc.semaphore("dma_out") as dout,
        nc.semaphore("act_sem") as act_sem,
        nc.semaphore("pe_sem") as pe_sem,
    ):
        dxs = [dx0, dx1, dx2, dx3, dx4]

        def x_dma(eng, ci, p0, p1):
            eng.dma_start(out=xin[ci][p0:p1, :],
                          in_=x_d[p0:p1, offs[ci]:offs[ci] + CHUNKS[ci]]
                          ).then_inc(dxs[ci], 16)

        def y_dma(eng, ci, req, p0=0, p1=P):
            eng.dma_start(out=y_d[p0:p1, offs[ci]:offs[ci] + CHUNKS[ci]],
                          in_=yout[ci][p0:p1, :]
                          )._wait_ge(act_sem, req).then_inc(dout, 16)

        # DMA rings are DESCRIPTOR-bound: a [128, anything] transfer costs
        # ~3.4us on the sync/scalar rings and ~8us on the gpsimd ring,
        # almost independent of column count (one descriptor per partition
        # line). So fill/tail-critical transfers (w, x0, x1, y4) are split
        # in HALF BY PARTITION across the sync+scalar rings (64
        # descriptors each ~1.7us); slack-rich bulk (x2, x4, y0-y2) rides
        # the slow gpsimd ring.
        H = P // 2

        @block.gpsimd
        def _(g: bass.BassEngine):
            # keep the slow gpsimd ring (and the shared fabric) clear of
            # the fill-critical w/x0/x1 window
            g.wait_ge(dxs[0], 32)
            x_dma(g, 2, H, P)
            g.wait_ge(dxs[1], 32)
            x_dma(g, 4, 0, P)
            y_dma(g, 0, idx[(5, 0)] + 1)
            y_dma(g, 1, idx[(5, 1)] + 1)
            y_dma(g, 3, idx[(5, 3)] + 1, H, P)
            # bottom-left quarter of the final chunk (cols 0:TH ready
            # after the first half of the split last tanh)
            g.dma_start(out=y_d[H:P, offs[4]:offs[4] + TH],
                        in_=yout[4][H:P, 0:TH]
                        )._wait_ge(act_sem, NST).then_inc(dout, 16)

        @block.sync
        def _(sync: bass.BassEngine):
            x_dma(sync, 0, 0, H)
            sync.dma_start(out=wall[0:H, :], in_=w_d[0:H, :]
                           ).then_inc(dww, 16)
            x_dma(sync, 1, 0, H)
            x_dma(sync, 2, 0, H)
            x_dma(sync, 3, 0, P)
            y_dma(sync, 2, idx[(5, 2)] + 1)
            y_dma(sync, 3, idx[(5, 3)] + 1, 0, H)
            # final chunk, top partitions: first-half columns as soon as
            # the split tanh's first half lands, then the second half
            sync.dma_start(out=y_d[0:H, offs[4]:offs[4] + TH],
                           in_=yout[4][0:H, 0:TH]
                           )._wait_ge(act_sem, NST).then_inc(dout, 16)
            sync.dma_start(out=y_d[0:H, offs[4] + TH:offs[4] + CHUNKS[4]],
                           in_=yout[4][0:H, TH:CHUNKS[4]]
                           )._wait_ge(act_sem, NST + 1).then_inc(dout, 16)
            sync.wait_ge(dout, 16 * 9)

        @block.scalar
        def _(scalar: bass.BassEngine):
            # bottom halves of x0/w/x1 ride the otherwise-idle ACT queue;
            # their issue cost lands during pipeline fill
            x_dma(scalar, 0, H, P)
            scalar.dma_start(out=wall[H:P, :], in_=w_d[H:P, :]
                             ).then_inc(dww, 16)
            x_dma(scalar, 1, H, P)
            for s, (kblk, ci) in enumerate(stages):
                if (kblk, ci) == LAST:
                    # split the final tanh by column so its output DMAs
                    # start ~1us earlier (they are partition-split
                    # quarters on the sync/gpsimd/scalar rings)
                    for a, b_ in ((0, TH), (TH, CHUNKS[ci])):
                        act = scalar.activation(
                            h_out(kblk, ci)[:, a:b_], ps[s % 2][:, a:b_],
                            Tanh, bias=0.0, scale=1.0)
                        if a == 0:
                            act._wait_ge(pe_sem, s + 1)
                        act.then_inc(act_sem, 1)
                else:
                    scalar.activation(
                        h_out(kblk, ci), ps[s % 2][:, 0:CHUNKS[ci]], Tanh,
                        bias=0.0, scale=1.0,
                    )._wait_ge(pe_sem, s + 1).then_inc(act_sem, 1)
            # bottom-right quarter of the final output on the now-idle
            # ACT queue (the act_sem wait makes the DMA observe the
            # tanh's posted SBUF writes - program order alone does not
            # order posted writes against another engine's reads)
            scalar.dma_start(
                out=y_d[H:P, offs[4] + TH:offs[4] + CHUNKS[4]],
                in_=yout[4][H:P, TH:CHUNKS[4]]
                )._wait_ge(act_sem, NST + 1).then_inc(dout, 16)

        @block.tensor
        def _(tensor: bass.BassEngine):
            tensor.wait_ge(dww, 32)
            for s, (kblk, ci) in enumerate(stages):
                ks = WSETS[kblk]
                if kblk == 0:
                    tensor.wait_ge(dxs[ci], 32 if ci <= 2 else 16)
                rhs = h_in(kblk, ci)
                for j, je, zone in MMPLAN[ci]:
                    kz = ks * 2 + zone
                    mm = tensor.matmul(ps[s % 2][:, j:je],
                                       wall[:, kz * P:(kz + 1) * P],
                                       rhs[:, j:je],
                                       start=True, stop=True)
                    if j == 0 and s >= 2:
                        # input-ready + psum WAR, folded into one wait
                        mm._wait_ge(act_sem, s - 1)
                mm.then_inc(pe_sem, 1)

    nc.compile()
    return nc


def _film_params(c, Wk, bk, Wsk, bsk, Wbk, bbk):
    """A[b] = diag(scale[b]) @ Wk ; d[b] = scale[b]*bk + shift[b], float64."""
    c = c.astype(np.float64)
    scale = 1.0 / (1.0 + np.exp(-(c @ Wsk.astype(np.float64).T
                                  + bsk.astype(np.float64))))     # [B,3]
    shift = c @ Wbk.astype(np.float64).T + bbk.astype(np.float64)  # [B,3]
    A = scale[:, :, None] * Wk.astype(np.float64)[None]            # [B,3,3]
    d = scale * bk.astype(np.float64) + shift                      # [B,3]
    return A, d


def kernel(t, x, c,
           W0, b0, Ws0, bs0, Wb0, bb0,
           W1, b1, Ws1, bs1, Wb1, bb1,
           W2, b2, Ws2, bs2, Wb2, bb2):
    global LAST_EXEC_NS
    if PROFILE:
        _install_profile_shim()
    from concourse.bass_utils import run_bass_kernel_spmd

    x = np.asarray(x)
    c = np.asarray(c)
    (W0, b0, Ws0, bs0, Wb0, bb0, W1, b1, Ws1, bs1, Wb1, bb1,
     W2, b2, Ws2, bs2, Wb2, bb2) = (
        np.asarray(a) for a in (W0, b0, Ws0, bs0, Wb0, bb0,
                                W1, b1, Ws1, bs1, Wb1, bb1,
                                W2, b2, Ws2, bs2, Wb2, bb2))
    out_dtype = x.dtype

    if "prog" not in _CACHE:
        _CACHE["prog"] = _build_program()
    nc = _CACHE["prog"]

    # ---- host: FiLM affine params per (weight-set, batch), float64 ----
    sets = [
        _film_params(c, W0, b0, Ws0, bs0, Wb0, bb0),
        _film_params(c, W1, b1, Ws1, bs1, Wb1, bb1),
        _film_params(c, W2, b2, Ws2, bs2, Wb2, bb2),
    ]

    # ---- host: shard + relayout x ----
    # [B, N, 3] -> per core [128, LC] fp16: stream t on partitions
    # 3t..3t+2, ones-row 126, zero-row 127. Stream table per core:
    # 10 full streams per batch + one shared boundary stream per batch
    # pair, switching batch at column BSPLIT.
    xp = np.ascontiguousarray(x, dtype=np.float32)
    xt = np.ascontiguousarray(xp.transpose(0, 2, 1))   # [B, 3, N]

    # (batch_lo, batch_hi, offset): full streams have lo == hi
    stream_table = []
    for pair in range(2):
        ba, bb = 2 * pair, 2 * pair + 1
        stream_table += [(ba, ba, t * LC) for t in range(10)]
        stream_table.append((ba, bb, NFULL))
        stream_table += [(bb, bb, t * LC) for t in range(10)]

    in_maps = []
    for cc in range(NCORES):
        b0 = cc * BPC
        X = np.zeros((P, LC), np.float16)
        for t, (blo, bhi, off) in enumerate(stream_table):
            for c_ in range(D):
                row = 3 * t + c_
                if blo == bhi:
                    X[row] = xt[b0 + blo, c_, off:off + LC]
                else:
                    X[row, :BSPLIT] = xt[b0 + blo, c_, NFULL:N]
                    X[row, BSPLIT:2 * BSPLIT] = xt[b0 + bhi, c_, NFULL:N]
        X[126] = 1.0                # ones-row: carries the bias via matmul
        W6 = np.zeros((P, 6 * P), np.float16)
        for k in range(3):
            A, dv = sets[k]
            for zone in range(2):
                c0 = (k * 2 + zone) * P
                for t, (blo, bhi, off) in enumerate(stream_table):
                    b = b0 + (blo if zone == 0 else bhi)
                    for ci_ in range(3):
                        for cj in range(3):
                            W6[3 * t + cj, c0 + 3 * t + ci_] = \
                                np.float16(A[b, ci_, cj])
                        # bias d rides the ones-row
                        W6[126, c0 + 3 * t + ci_] = np.float16(dv[b, ci_])
                # ones-row regenerates itself: tanh(16.0) == 1.0 in fp16
                W6[126, c0 + 126] = np.float16(16.0)
        in_maps.append({"x": X, "w": W6})

    res = run_bass_kernel_spmd(nc, in_maps, list(range(NCORES)),
                               trace=bool(PROFILE))
    if PROFILE:
        LAST_EXEC_NS = res.exec_time_ns

    # ---- host: gather + inverse layout ----
    yt = np.empty((B, D, N), np.float32)
    for cc in range(NCORES):
        Y = res.results[cc]["y"]                       # [P, LC] fp16
        b0 = cc * BPC
        for t, (blo, bhi, off) in enumerate(stream_table):
            for c_ in range(D):
                row = 3 * t + c_
                if blo == bhi:
                    yt[b0 + blo, c_, off:off + LC] = Y[row]
                else:
                    yt[b0 + blo, c_, NFULL:N] = Y[row, :BSPLIT]
                    yt[b0 + bhi, c_, NFULL:N] = Y[row, BSPLIT:2 * BSPLIT]
    out = np.ascontiguousarray(yt.transpose(0, 2, 1)).astype(
        out_dtype, copy=False)
    return out
